# Initial kernel scaffold
#
"""Trainium2 Bass kernel for a text-adapter block (LN -> 768->16 -> ReLU ->
16->768 -> *0.1 -> +residual), data-parallel over 8 NeuronCores.

Self-contained: takes the FULL inputs from setup_inputs(), shards x on the
token axis across 8 cores, runs one compiled Bass module SPMD, gathers.

Math (exact, with LN affine folded into the adapter weights on the host):
  W' = gamma[:,None] * w_down          b' = b_down + beta @ w_down
  c  = colsum(W')                      psi[k,t] = sum_d x[d,t] W'[d,k]
                                                - mean_t * c[k] + std_t * b'[k]
  down[k,t] = rstd_t * relu(psi[k,t])          (relu commutes with rstd>0)
  pup[t,d]  = sum_k relu(psi)[k,t] * (0.1*w_up)[k,d] + std_t * (0.1*b_up)[d]
  out[t,d]  = rstd_t * pup[t,d] + x[t,d]
"""

import numpy as np

D_MODEL = 768
BOTTLENECK = 16
SCALE = 0.1
LN_EPS = 1e-5
P = 128
N_CORES = 8
CHUNKS = D_MODEL // P  # 6

# Columns of the final (pup*rstd + x) handled by DVE scalar_tensor_tensor
# directly from PSUM; the rest is evacuated by ACT (Copy*rstd) and the
# residual added on GPSIMD.  Tunable for engine balance.
FINAL_DVE_COLS = 256

_CACHE: dict = {}


def _build(rows_per_core: int):
    from contextlib import ExitStack

    import concourse.bacc as bacc
    import concourse.tile as tile
    from concourse import mybir

    nc = bacc.Bacc(
        "TRN2",
        target_bir_lowering=False,
        debug=False,
        enable_asserts=False,
        num_devices=N_CORES,
    )
    f32 = mybir.dt.float32
    bf16 = mybir.dt.bfloat16

    x_d = nc.dram_tensor("x", [rows_per_core, D_MODEL], f32, kind="ExternalInput").ap()
    wd_d = nc.dram_tensor("wd", [D_MODEL, BOTTLENECK], bf16, kind="ExternalInput").ap()
    cb_d = nc.dram_tensor("cb", [2, BOTTLENECK], bf16, kind="ExternalInput").ap()
    wu_d = nc.dram_tensor("wu", [BOTTLENECK, D_MODEL], bf16, kind="ExternalInput").ap()
    bubx_d = nc.dram_tensor("bubx", [2, D_MODEL], bf16, kind="ExternalInput").ap()
    ident_d = nc.dram_tensor("ident", [P, P], f32, kind="ExternalInput").ap()
    out_d = nc.dram_tensor(
        "out", [rows_per_core, D_MODEL], f32, kind="ExternalOutput"
    ).ap()

    ntiles = rows_per_core // P
    S = FINAL_DVE_COLS
    Relu = mybir.ActivationFunctionType.Relu
    Sqrt = mybir.ActivationFunctionType.Sqrt
    Copy = mybir.ActivationFunctionType.Copy
    mult = mybir.AluOpType.mult
    add = mybir.AluOpType.add

    with tile.TileContext(nc) as tc, ExitStack() as ctx:
        consts = ctx.enter_context(tc.tile_pool(name="consts", bufs=1))
        xpool = ctx.enter_context(tc.tile_pool(name="xpool", bufs=4))
        opool = ctx.enter_context(tc.tile_pool(name="opool", bufs=3))
        xtpool = ctx.enter_context(tc.tile_pool(name="xtpool", bufs=2))
        uppool = ctx.enter_context(tc.tile_pool(name="uppool", bufs=2))
        tiny = ctx.enter_context(tc.tile_pool(name="tiny", bufs=4))
        p_xt = ctx.enter_context(tc.tile_pool(name="p_xt", bufs=1, space="PSUM"))
        p_psi = ctx.enter_context(tc.tile_pool(name="p_psi", bufs=2, space="PSUM"))
        p_up = ctx.enter_context(tc.tile_pool(name="p_up", bufs=2, space="PSUM"))

        # ---- constants (loaded once) ----
        wd_sb = consts.tile([P, CHUNKS, BOTTLENECK], bf16)
        nc.sync.dma_start(
            out=wd_sb, in_=wd_d.rearrange("(c p) k -> p c k", p=P)
        )
        cb_sb = consts.tile([2, BOTTLENECK], bf16)
        nc.sync.dma_start(out=cb_sb, in_=cb_d)
        wu_sb = consts.tile([BOTTLENECK, D_MODEL], bf16)
        nc.sync.dma_start(out=wu_sb, in_=wu_d)
        bubx_sb = consts.tile([2, D_MODEL], bf16)
        nc.sync.dma_start(out=bubx_sb, in_=bubx_d)
        ident_sb = consts.tile([P, P], f32)
        nc.sync.dma_start(out=ident_sb, in_=ident_d)

        for i in range(ntiles):
            r0 = i * P
            x_sb = xpool.tile([P, D_MODEL], f32)
            nc.sync.dma_start(out=x_sb, in_=x_d[r0 : r0 + P, :])

            # ---- LayerNorm stats (token-major) ----
            st6 = tiny.tile([P, 2, 6], f32, tag="st6")
            nc.vector.bn_stats(out=st6[:, 0, :], in_=x_sb[:, 0 : D_MODEL // 2])
            nc.vector.bn_stats(out=st6[:, 1, :], in_=x_sb[:, D_MODEL // 2 :])
            mv = tiny.tile([P, 2], f32, tag="mv")  # (mean, var) -> (mean, std)
            nc.vector.bn_aggr(out=mv, in_=st6)
            nc.scalar.activation(
                out=mv[:, 1:2], in_=mv[:, 1:2], func=Sqrt, bias=LN_EPS
            )
            rstd = tiny.tile([P, 1], f32, tag="rstd")
            nc.vector.reciprocal(out=rstd, in_=mv[:, 1:2])
            packb = tiny.tile([P, 2], bf16, tag="packb")
            nc.vector.tensor_copy(out=packb, in_=mv)
            # (mean, std) to free-major [2, P] via two tiny DMA transposes
            msrow = tiny.tile([2, P], bf16, tag="msrow")
            nc.sync.dma_start(out=msrow[0:1, :], in_=packb[:, 0:1])
            nc.sync.dma_start(out=msrow[1:2, :], in_=packb[:, 1:2])

            # ---- transpose raw x to feature-major (PE), evac to bf16 ----
            pxt = p_xt.tile([P, D_MODEL], f32)
            for c in range(CHUNKS):
                nc.tensor.transpose(
                    out=pxt[:, c * P : (c + 1) * P],
                    in_=x_sb[:, c * P : (c + 1) * P],
                    identity=ident_sb,
                )
            xt_b16 = xtpool.tile([P, D_MODEL], bf16)
            nc.scalar.activation(out=xt_b16, in_=pxt, func=Copy)

            # ---- down-proj: psi[k,t] in PSUM [16, 128] ----
            ppsi = p_psi.tile([BOTTLENECK, P], f32)
            for c in range(CHUNKS):
                nc.tensor.matmul(
                    ppsi,
                    lhsT=wd_sb[:, c, :],
                    rhs=xt_b16[:, c * P : (c + 1) * P],
                    start=(c == 0),
                    stop=False,
                )
            # corrections: -c (x) mean + b' (x) std   (K=2 rank-2 update)
            nc.tensor.matmul(ppsi, lhsT=cb_sb, rhs=msrow, start=False, stop=True)

            relu_b16 = tiny.tile([BOTTLENECK, P], bf16, tag="relu")
            nc.scalar.activation(out=relu_b16, in_=ppsi, func=Relu)

            # ---- up-proj + bias into PSUM [128, 768] (two banks) ----
            pup = p_up.tile([P, D_MODEL], f32)
            for lo, hi in ((0, 512), (512, D_MODEL)):
                nc.tensor.matmul(
                    pup[:, lo:hi],
                    lhsT=relu_b16,
                    rhs=wu_sb[:, lo:hi],
                    start=True,
                    stop=False,
                )
                # + std (x) bub   (row 0 of bubx is zeros, pairs with mean)
                nc.tensor.matmul(
                    pup[:, lo:hi],
                    lhsT=msrow,
                    rhs=bubx_sb[:, lo:hi],
                    start=False,
                    stop=True,
                )

            # ---- final: out = pup * rstd + x ----
            out_sb = opool.tile([P, D_MODEL], f32)
            if S > 0:
                nc.vector.scalar_tensor_tensor(
                    out=out_sb[:, 0:S],
                    in0=pup[:, 0:S],
                    scalar=rstd,
                    in1=x_sb[:, 0:S],
                    op0=mult,
                    op1=add,
                )
            if S < D_MODEL:
                up_sb = uppool.tile([P, D_MODEL - S], f32)
                nc.scalar.activation(
                    out=up_sb, in_=pup[:, S:], func=Copy, scale=rstd
                )
                nc.gpsimd.tensor_add(out_sb[:, S:], up_sb, x_sb[:, S:])

            nc.sync.dma_start(out=out_d[r0 : r0 + P, :], in_=out_sb)

    nc.compile()
    return nc


def _get_nc(rows_per_core: int):
    if rows_per_core not in _CACHE:
        _CACHE[rows_per_core] = _build(rows_per_core)
    return _CACHE[rows_per_core]


def _host_consts(ln_gamma, ln_beta, w_down, b_down, w_up, b_up):
    import ml_dtypes

    bf = ml_dtypes.bfloat16
    ln_gamma = np.asarray(ln_gamma, np.float32)
    ln_beta = np.asarray(ln_beta, np.float32)
    w_down = np.asarray(w_down, np.float32)
    b_down = np.asarray(b_down, np.float32)
    w_up = np.asarray(w_up, np.float32)
    b_up = np.asarray(b_up, np.float32)

    wd_eff = ln_gamma[:, None] * w_down
    b_eff = b_down + ln_beta @ w_down
    cb = np.stack([-wd_eff.sum(0), b_eff]).astype(bf)
    bubx = np.stack([np.zeros(D_MODEL, np.float32), SCALE * b_up]).astype(bf)
    return dict(
        wd=wd_eff.astype(bf),
        cb=np.ascontiguousarray(cb),
        wu=(SCALE * w_up).astype(bf),
        bubx=np.ascontiguousarray(bubx),
        ident=np.eye(P, dtype=np.float32),
    )


def kernel(x, ln_gamma, ln_beta, w_down, b_down, w_up, b_up):
    from concourse.bass_utils import run_bass_kernel_spmd

    x = np.asarray(x, np.float32)
    b, t, d = x.shape
    rows = b * t
    rpc = rows // N_CORES
    consts = _host_consts(ln_gamma, ln_beta, w_down, b_down, w_up, b_up)
    xf = x.reshape(rows, d)
    in_maps = [
        dict(x=np.ascontiguousarray(xf[i * rpc : (i + 1) * rpc]), **consts)
        for i in range(N_CORES)
    ]
    nc = _get_nc(rpc)
    res = run_bass_kernel_spmd(nc, in_maps, core_ids=list(range(N_CORES)))
    out = np.concatenate([r["out"] for r in res.results], axis=0)
    return np.ascontiguousarray(out.reshape(b, t, d).astype(np.float32))


# revision 6
# speedup vs baseline: 63168.4493x; 63168.4493x over previous
"""Trainium2 Bass kernel for a text-adapter block (LN -> 768->16 -> ReLU ->
16->768 -> *0.1 -> +residual), data-parallel over 8 NeuronCores.

Self-contained: takes the FULL inputs from setup_inputs(), shards x on the
token axis across 8 cores, runs one compiled Bass module SPMD, gathers.

Math (exact, with LN affine folded into the adapter weights on the host):
  W' = gamma[:,None] * w_down          b' = b_down + beta @ w_down
  c  = colsum(W')                      psi[k,t] = sum_d x[d,t] W'[d,k]
                                                - mean_t * c[k] + std_t * b'[k]
  down[k,t] = rstd_t * relu(psi[k,t])          (relu commutes with rstd>0)
  pup[t,d]  = sum_k relu(psi)[k,t] * (0.1*w_up)[k,d] + std_t * (0.1*b_up)[d]
  out[t,d]  = rstd_t * pup[t,d] + x[t,d]
"""

import numpy as np

D_MODEL = 768
BOTTLENECK = 16
SCALE = 0.1
LN_EPS = 1e-5
P = 128
N_CORES = 8
CHUNKS = D_MODEL // P  # 6

# Columns of the final (pup*rstd + x) handled by DVE scalar_tensor_tensor
# directly from PSUM; the rest is evacuated by ACT (Copy*rstd) and the
# residual added on GPSIMD.  Tunable for engine balance.
FINAL_DVE_COLS = 256

_CACHE: dict = {}


def _build(rows_per_core: int, reps: int = 1):
    from contextlib import ExitStack

    import concourse.bacc as bacc
    import concourse.tile as tile
    from concourse import mybir

    nc = bacc.Bacc(
        "TRN2",
        target_bir_lowering=False,
        debug=False,
        enable_asserts=False,
        num_devices=N_CORES,
    )
    f32 = mybir.dt.float32
    bf16 = mybir.dt.bfloat16

    x_d = nc.dram_tensor("x", [rows_per_core, D_MODEL], f32, kind="ExternalInput").ap()
    wd_d = nc.dram_tensor("wd", [D_MODEL, BOTTLENECK], bf16, kind="ExternalInput").ap()
    cb_d = nc.dram_tensor("cb", [2, BOTTLENECK], bf16, kind="ExternalInput").ap()
    wu_d = nc.dram_tensor("wu", [BOTTLENECK, D_MODEL], bf16, kind="ExternalInput").ap()
    bubx_d = nc.dram_tensor("bubx", [2, D_MODEL], bf16, kind="ExternalInput").ap()
    ident_d = nc.dram_tensor("ident", [P, P], f32, kind="ExternalInput").ap()
    out_d = nc.dram_tensor(
        "out", [rows_per_core, D_MODEL], f32, kind="ExternalOutput"
    ).ap()

    ntiles = rows_per_core // P
    S = FINAL_DVE_COLS
    Relu = mybir.ActivationFunctionType.Relu
    Sqrt = mybir.ActivationFunctionType.Sqrt
    Copy = mybir.ActivationFunctionType.Copy
    mult = mybir.AluOpType.mult
    add = mybir.AluOpType.add

    with tile.TileContext(nc) as tc, ExitStack() as ctx:
        consts = ctx.enter_context(tc.tile_pool(name="consts", bufs=1))
        xpool = ctx.enter_context(tc.tile_pool(name="xpool", bufs=4))
        opool = ctx.enter_context(tc.tile_pool(name="opool", bufs=3))
        xtpool = ctx.enter_context(tc.tile_pool(name="xtpool", bufs=2))
        uppool = ctx.enter_context(tc.tile_pool(name="uppool", bufs=2))
        tiny = ctx.enter_context(tc.tile_pool(name="tiny", bufs=4))
        p_xt = ctx.enter_context(tc.tile_pool(name="p_xt", bufs=1, space="PSUM"))
        p_psi = ctx.enter_context(tc.tile_pool(name="p_psi", bufs=2, space="PSUM"))
        p_up = ctx.enter_context(tc.tile_pool(name="p_up", bufs=2, space="PSUM"))

        # ---- constants (loaded once) ----
        wd_sb = consts.tile([P, CHUNKS, BOTTLENECK], bf16)
        nc.sync.dma_start(
            out=wd_sb, in_=wd_d.rearrange("(c p) k -> p c k", p=P)
        )
        cb_sb = consts.tile([2, BOTTLENECK], bf16)
        nc.sync.dma_start(out=cb_sb, in_=cb_d)
        wu_sb = consts.tile([BOTTLENECK, D_MODEL], bf16)
        nc.sync.dma_start(out=wu_sb, in_=wu_d)
        bubx_sb = consts.tile([2, D_MODEL], bf16)
        nc.sync.dma_start(out=bubx_sb, in_=bubx_d)
        ident_sb = consts.tile([P, P], f32)
        nc.sync.dma_start(out=ident_sb, in_=ident_d)
        eps_sb = consts.tile([P, 1], f32)
        nc.vector.memset(eps_sb, LN_EPS)

        for i in range(ntiles * reps):
            r0 = (i % ntiles) * P
            x_sb = xpool.tile([P, D_MODEL], f32)
            nc.sync.dma_start(out=x_sb, in_=x_d[r0 : r0 + P, :])

            # ---- LayerNorm stats (token-major) ----
            st6 = tiny.tile([P, 2, 6], f32, tag="st6")
            nc.vector.bn_stats(out=st6[:, 0, :], in_=x_sb[:, 0 : D_MODEL // 2])
            nc.vector.bn_stats(out=st6[:, 1, :], in_=x_sb[:, D_MODEL // 2 :])
            mv = tiny.tile([P, 2], f32, tag="mv")  # (mean, var) -> (mean, std)
            nc.vector.bn_aggr(out=mv, in_=st6)
            nc.scalar.activation(
                out=mv[:, 1:2], in_=mv[:, 1:2], func=Sqrt, bias=eps_sb
            )
            rstd = tiny.tile([P, 1], f32, tag="rstd")
            nc.vector.reciprocal(out=rstd, in_=mv[:, 1:2])
            packb = tiny.tile([P, 2], bf16, tag="packb")
            nc.vector.tensor_copy(out=packb, in_=mv)
            # (mean, std) to free-major [2, P] via two tiny DMA transposes
            msrow = tiny.tile([2, P], bf16, tag="msrow")
            nc.sync.dma_start(out=msrow[0:1, :], in_=packb[:, 0:1])
            nc.sync.dma_start(out=msrow[1:2, :], in_=packb[:, 1:2])

            # ---- transpose raw x to feature-major (PE), evac to bf16 ----
            pxt = p_xt.tile([P, D_MODEL], f32)
            for c in range(CHUNKS):
                nc.tensor.transpose(
                    out=pxt[:, c * P : (c + 1) * P],
                    in_=x_sb[:, c * P : (c + 1) * P],
                    identity=ident_sb,
                )
            xt_b16 = xtpool.tile([P, D_MODEL], bf16)
            nc.scalar.activation(out=xt_b16, in_=pxt, func=Copy)

            # ---- down-proj: psi[k,t] in PSUM [16, 128] ----
            ppsi = p_psi.tile([BOTTLENECK, P], f32)
            for c in range(CHUNKS):
                nc.tensor.matmul(
                    ppsi,
                    lhsT=wd_sb[:, c, :],
                    rhs=xt_b16[:, c * P : (c + 1) * P],
                    start=(c == 0),
                    stop=False,
                )
            # corrections: -c (x) mean + b' (x) std   (K=2 rank-2 update)
            nc.tensor.matmul(ppsi, lhsT=cb_sb, rhs=msrow, start=False, stop=True)

            relu_b16 = tiny.tile([BOTTLENECK, P], bf16, tag="relu")
            nc.scalar.activation(out=relu_b16, in_=ppsi, func=Relu)

            # ---- up-proj + bias into PSUM [128, 768] (two banks) ----
            pup = p_up.tile([P, D_MODEL], f32)
            for lo, hi in ((0, 512), (512, D_MODEL)):
                nc.tensor.matmul(
                    pup[:, lo:hi],
                    lhsT=relu_b16,
                    rhs=wu_sb[:, lo:hi],
                    start=True,
                    stop=False,
                )
                # + std (x) bub   (row 0 of bubx is zeros, pairs with mean)
                nc.tensor.matmul(
                    pup[:, lo:hi],
                    lhsT=msrow,
                    rhs=bubx_sb[:, lo:hi],
                    start=False,
                    stop=True,
                )

            # ---- final: out = pup * rstd + x ----
            out_sb = opool.tile([P, D_MODEL], f32)
            if S > 0:
                nc.vector.scalar_tensor_tensor(
                    out=out_sb[:, 0:S],
                    in0=pup[:, 0:S],
                    scalar=rstd,
                    in1=x_sb[:, 0:S],
                    op0=mult,
                    op1=add,
                )
            if S < D_MODEL:
                up_sb = uppool.tile([P, D_MODEL - S], f32)
                nc.scalar.activation(
                    out=up_sb, in_=pup[:, S:], func=Copy, scale=rstd
                )
                nc.gpsimd.tensor_add(out_sb[:, S:], up_sb, x_sb[:, S:])

            nc.sync.dma_start(out=out_d[r0 : r0 + P, :], in_=out_sb)

    nc.compile()
    return nc


def _get_nc(rows_per_core: int, reps: int = 1):
    key = (rows_per_core, reps)
    if key not in _CACHE:
        _CACHE[key] = _build(rows_per_core, reps)
    return _CACHE[key]


def _host_consts(ln_gamma, ln_beta, w_down, b_down, w_up, b_up):
    import ml_dtypes

    bf = ml_dtypes.bfloat16
    ln_gamma = np.asarray(ln_gamma, np.float32)
    ln_beta = np.asarray(ln_beta, np.float32)
    w_down = np.asarray(w_down, np.float32)
    b_down = np.asarray(b_down, np.float32)
    w_up = np.asarray(w_up, np.float32)
    b_up = np.asarray(b_up, np.float32)

    wd_eff = ln_gamma[:, None] * w_down
    b_eff = b_down + ln_beta @ w_down
    cb = np.stack([-wd_eff.sum(0), b_eff]).astype(bf)
    bubx = np.stack([np.zeros(D_MODEL, np.float32), SCALE * b_up]).astype(bf)
    return dict(
        wd=wd_eff.astype(bf),
        cb=np.ascontiguousarray(cb),
        wu=(SCALE * w_up).astype(bf),
        bubx=np.ascontiguousarray(bubx),
        ident=np.eye(P, dtype=np.float32),
    )


def kernel(x, ln_gamma, ln_beta, w_down, b_down, w_up, b_up):
    from concourse.bass_utils import run_bass_kernel_spmd

    x = np.asarray(x, np.float32)
    b, t, d = x.shape
    rows = b * t
    rpc = rows // N_CORES
    consts = _host_consts(ln_gamma, ln_beta, w_down, b_down, w_up, b_up)
    xf = x.reshape(rows, d)
    in_maps = [
        dict(x=np.ascontiguousarray(xf[i * rpc : (i + 1) * rpc]), **consts)
        for i in range(N_CORES)
    ]
    nc = _get_nc(rpc)
    res = run_bass_kernel_spmd(nc, in_maps, core_ids=list(range(N_CORES)))
    out = np.concatenate([r["out"] for r in res.results], axis=0)
    return np.ascontiguousarray(out.reshape(b, t, d).astype(np.float32))


# revision 8
# speedup vs baseline: 198345.3769x; 3.1399x over previous
"""Trainium2 Bass kernel for a text-adapter block (LN -> 768->16 -> ReLU ->
16->768 -> *0.1 -> +residual), data-parallel over 8 NeuronCores.

Self-contained: takes the FULL inputs from setup_inputs(), shards x on the
token axis across 8 cores, runs one compiled Bass module SPMD, gathers.

Math (exact, with LN affine folded into the adapter weights on the host):
  W' = gamma[:,None] * w_down          b' = b_down + beta @ w_down
  c  = colsum(W')                      psi[k,t] = sum_d x[d,t] W'[d,k]
                                                - mean_t * c[k] + std_t * b'[k]
  down[k,t] = rstd_t * relu(psi[k,t])          (relu commutes with rstd>0)
  pup[t,d]  = sum_k relu(psi)[k,t] * (0.1*w_up)[k,d] + std_t * (0.1*b_up)[d]
  out[t,d]  = rstd_t * pup[t,d] + x[t,d]
"""

import numpy as np

D_MODEL = 768
BOTTLENECK = 16
SCALE = 0.1
LN_EPS = 1e-5
P = 128
N_CORES = 8
CHUNKS = D_MODEL // P  # 6

# Columns of the final (pup*rstd + x) handled by DVE scalar_tensor_tensor
# directly from PSUM; the rest is evacuated by ACT (Copy*rstd) and the
# residual added on GPSIMD.  Tunable for engine balance.
FINAL_DVE_COLS = 256

_CACHE: dict = {}


def _build(rows_per_core: int, reps: int = 1):
    from contextlib import ExitStack

    import concourse.bacc as bacc
    import concourse.tile as tile
    from concourse import mybir

    nc = bacc.Bacc(
        "TRN2",
        target_bir_lowering=False,
        debug=False,
        enable_asserts=False,
        num_devices=N_CORES,
    )
    f32 = mybir.dt.float32
    bf16 = mybir.dt.bfloat16

    x_d = nc.dram_tensor("x", [rows_per_core, D_MODEL], f32, kind="ExternalInput").ap()
    wd_d = nc.dram_tensor("wd", [D_MODEL, BOTTLENECK], bf16, kind="ExternalInput").ap()
    cb_d = nc.dram_tensor("cb", [2, BOTTLENECK], bf16, kind="ExternalInput").ap()
    wu_d = nc.dram_tensor("wu", [BOTTLENECK, D_MODEL], bf16, kind="ExternalInput").ap()
    bubx_d = nc.dram_tensor("bubx", [2, D_MODEL], bf16, kind="ExternalInput").ap()
    ident_d = nc.dram_tensor("ident", [P, P], f32, kind="ExternalInput").ap()
    out_d = nc.dram_tensor(
        "out", [rows_per_core, D_MODEL], f32, kind="ExternalOutput"
    ).ap()

    ntiles = rows_per_core // P
    S = FINAL_DVE_COLS
    Relu = mybir.ActivationFunctionType.Relu
    Sqrt = mybir.ActivationFunctionType.Sqrt
    Copy = mybir.ActivationFunctionType.Copy
    mult = mybir.AluOpType.mult
    add = mybir.AluOpType.add

    with tile.TileContext(nc) as tc, ExitStack() as ctx:
        consts = ctx.enter_context(tc.tile_pool(name="consts", bufs=1))
        xpool = ctx.enter_context(tc.tile_pool(name="xpool", bufs=4))
        opool = ctx.enter_context(tc.tile_pool(name="opool", bufs=3))
        xtpool = ctx.enter_context(tc.tile_pool(name="xtpool", bufs=2))
        uppool = ctx.enter_context(tc.tile_pool(name="uppool", bufs=2))
        tiny = ctx.enter_context(tc.tile_pool(name="tiny", bufs=4))
        p_xt = ctx.enter_context(tc.tile_pool(name="p_xt", bufs=1, space="PSUM"))
        p_psi = ctx.enter_context(tc.tile_pool(name="p_psi", bufs=1, space="PSUM"))
        p_st = ctx.enter_context(tc.tile_pool(name="p_st", bufs=1, space="PSUM"))
        p_up = ctx.enter_context(tc.tile_pool(name="p_up", bufs=2, space="PSUM"))

        # ---- constants (loaded once) ----
        wd_sb = consts.tile([P, CHUNKS, BOTTLENECK], bf16)
        nc.sync.dma_start(
            out=wd_sb, in_=wd_d.rearrange("(c p) k -> p c k", p=P)
        )
        cb_sb = consts.tile([2, BOTTLENECK], bf16)
        nc.sync.dma_start(out=cb_sb, in_=cb_d)
        wu_sb = consts.tile([BOTTLENECK, D_MODEL], bf16)
        nc.sync.dma_start(out=wu_sb, in_=wu_d)
        bubx_sb = consts.tile([2, D_MODEL], bf16)
        nc.sync.dma_start(out=bubx_sb, in_=bubx_d)
        ident_sb = consts.tile([P, P], f32)
        nc.sync.dma_start(out=ident_sb, in_=ident_d)
        eps_sb = consts.tile([P, 1], f32)
        nc.vector.memset(eps_sb, LN_EPS)

        for i in range(ntiles * reps):
            r0 = (i % ntiles) * P
            x_sb = xpool.tile([P, D_MODEL], f32)
            nc.sync.dma_start(out=x_sb, in_=x_d[r0 : r0 + P, :])

            # ---- LayerNorm stats (token-major) ----
            st6 = tiny.tile([P, 2, 6], f32, tag="st6")
            nc.vector.bn_stats(out=st6[:, 0, :], in_=x_sb[:, 0 : D_MODEL // 2])
            nc.vector.bn_stats(out=st6[:, 1, :], in_=x_sb[:, D_MODEL // 2 :])
            mv = tiny.tile([P, 2], f32, tag="mv")  # (mean, var) -> (mean, std)
            nc.vector.bn_aggr(out=mv, in_=st6)
            nc.scalar.activation(
                out=mv[:, 1:2], in_=mv[:, 1:2], func=Sqrt, bias=eps_sb
            )
            rstd = tiny.tile([P, 1], f32, tag="rstd")
            nc.vector.reciprocal(out=rstd, in_=mv[:, 1:2])
            # (mean, std) to free-major [2, P]: PE transpose + DVE evac
            pstat = p_st.tile([2, P], f32)
            nc.tensor.transpose(out=pstat, in_=mv, identity=ident_sb)
            msrow = tiny.tile([2, P], bf16, tag="msrow")
            nc.vector.tensor_copy(out=msrow, in_=pstat)

            # ---- transpose raw x to feature-major (PE), evac to bf16 ----
            pxt = p_xt.tile([P, D_MODEL], f32)
            for c in range(CHUNKS):
                nc.tensor.transpose(
                    out=pxt[:, c * P : (c + 1) * P],
                    in_=x_sb[:, c * P : (c + 1) * P],
                    identity=ident_sb,
                )
            xt_b16 = xtpool.tile([P, D_MODEL], bf16)
            nc.scalar.activation(out=xt_b16, in_=pxt, func=Copy)

            # ---- down-proj: psi[k,t] in PSUM [16, 128] ----
            ppsi = p_psi.tile([BOTTLENECK, P], f32)
            for c in range(CHUNKS):
                nc.tensor.matmul(
                    ppsi,
                    lhsT=wd_sb[:, c, :],
                    rhs=xt_b16[:, c * P : (c + 1) * P],
                    start=(c == 0),
                    stop=False,
                )
            # corrections: -c (x) mean + b' (x) std   (K=2 rank-2 update)
            nc.tensor.matmul(ppsi, lhsT=cb_sb, rhs=msrow, start=False, stop=True)

            relu_b16 = tiny.tile([BOTTLENECK, P], bf16, tag="relu")
            nc.scalar.activation(out=relu_b16, in_=ppsi, func=Relu)

            # ---- up-proj + bias into PSUM [128, 768] (two banks) ----
            pup = p_up.tile([P, D_MODEL], f32)
            for lo, hi in ((0, 512), (512, D_MODEL)):
                nc.tensor.matmul(
                    pup[:, lo:hi],
                    lhsT=relu_b16,
                    rhs=wu_sb[:, lo:hi],
                    start=True,
                    stop=False,
                )
                # + std (x) bub   (row 0 of bubx is zeros, pairs with mean)
                nc.tensor.matmul(
                    pup[:, lo:hi],
                    lhsT=msrow,
                    rhs=bubx_sb[:, lo:hi],
                    start=False,
                    stop=True,
                )

            # ---- final: out = pup * rstd + x ----
            out_sb = opool.tile([P, D_MODEL], f32)
            if S > 0:
                nc.vector.scalar_tensor_tensor(
                    out=out_sb[:, 0:S],
                    in0=pup[:, 0:S],
                    scalar=rstd,
                    in1=x_sb[:, 0:S],
                    op0=mult,
                    op1=add,
                )
            if S < D_MODEL:
                up_sb = uppool.tile([P, D_MODEL - S], f32)
                nc.scalar.activation(
                    out=up_sb, in_=pup[:, S:], func=Copy, scale=rstd
                )
                nc.gpsimd.tensor_add(out_sb[:, S:], up_sb, x_sb[:, S:])

            nc.sync.dma_start(out=out_d[r0 : r0 + P, :], in_=out_sb)

    nc.compile()
    return nc


def _get_nc(rows_per_core: int, reps: int = 1):
    key = (rows_per_core, reps)
    if key not in _CACHE:
        _CACHE[key] = _build(rows_per_core, reps)
    return _CACHE[key]


def _host_consts(ln_gamma, ln_beta, w_down, b_down, w_up, b_up):
    import ml_dtypes

    bf = ml_dtypes.bfloat16
    ln_gamma = np.asarray(ln_gamma, np.float32)
    ln_beta = np.asarray(ln_beta, np.float32)
    w_down = np.asarray(w_down, np.float32)
    b_down = np.asarray(b_down, np.float32)
    w_up = np.asarray(w_up, np.float32)
    b_up = np.asarray(b_up, np.float32)

    wd_eff = ln_gamma[:, None] * w_down
    b_eff = b_down + ln_beta @ w_down
    cb = np.stack([-wd_eff.sum(0), b_eff]).astype(bf)
    bubx = np.stack([np.zeros(D_MODEL, np.float32), SCALE * b_up]).astype(bf)
    return dict(
        wd=wd_eff.astype(bf),
        cb=np.ascontiguousarray(cb),
        wu=(SCALE * w_up).astype(bf),
        bubx=np.ascontiguousarray(bubx),
        ident=np.eye(P, dtype=np.float32),
    )


def kernel(x, ln_gamma, ln_beta, w_down, b_down, w_up, b_up):
    from concourse.bass_utils import run_bass_kernel_spmd

    x = np.asarray(x, np.float32)
    b, t, d = x.shape
    rows = b * t
    rpc = rows // N_CORES
    consts = _host_consts(ln_gamma, ln_beta, w_down, b_down, w_up, b_up)
    xf = x.reshape(rows, d)
    in_maps = [
        dict(x=np.ascontiguousarray(xf[i * rpc : (i + 1) * rpc]), **consts)
        for i in range(N_CORES)
    ]
    nc = _get_nc(rpc)
    res = run_bass_kernel_spmd(nc, in_maps, core_ids=list(range(N_CORES)))
    out = np.concatenate([r["out"] for r in res.results], axis=0)
    return np.ascontiguousarray(out.reshape(b, t, d).astype(np.float32))
